# revision 57
# baseline (speedup 1.0000x reference)
"""Multi-head attention (B=4, N=2048, D=1024, H=16) on 8 Trainium2 NeuronCores.

Sharding: core c = 2*b + hg handles batch b and head-group hg (8 of 16 heads).
Host pre-transposes x and mask per batch, slices Wq/Wk/Wv columns and Wo rows
per head group, and sums the two partial outputs per batch (+ bo) at the end.

Per-core kernel, per-kc (128 keys) software pipeline:
  scores (PE, 2 heads row-packed) -> mask-mult (DVE, psum->sbuf f16, frees
  psum after ONE pass) -> exp (ACT, 2-kc tiles) -> ctx (PE, 2 heads
  col-tiled into one 128-row psum bank) + denominator (PE, 2 col-packed
  M=1 matmuls).  Per-pair normalize: ln/exp on the two denominator rows, a
  selector-matrix broadcast matmul, one ACT copy, one DVE mult.
  qk projections for pairs 1-3 are emitted inside the first q-chunk's rows
  so they overlap the DVE/ACT-paced attention pipeline.
"""
import os
from contextlib import ExitStack

import numpy as np

from concourse import bacc, mybir, tile
from concourse import bass_utils

P = 128
NSEQ = 2048          # sequence length
DMODEL = 1024        # model dim
HD = 512             # per-core head dim total (8 heads x 64)
NPAIR = 4            # head pairs per core
DH = 64              # head depth
DC = DMODEL // P     # 8 d_model chunks
NQC = 4              # q chunks of 512
NKC = 16             # k chunks of 128
F32 = mybir.dt.float32
F32R = mybir.dt.float32r
F16 = mybir.dt.float16
F8 = mybir.dt.float8e4
DR = mybir.MatmulPerfMode.DoubleRow
AF = mybir.ActivationFunctionType
OP = mybir.AluOpType

_CACHE: dict = {}
LAST_RESULTS = None

# crash-bisect flags (default: all features on)
_NO_DENOM = bool(os.environ.get("KB_NO_DENOM"))
_NO_BCMM2 = bool(os.environ.get("KB_NO_BCMM2"))
_NO_NORM = bool(os.environ.get("KB_NO_NORM"))
_NO_MKBCAST = bool(os.environ.get("KB_NO_MKBCAST"))
_NO_QKINT = bool(os.environ.get("KB_NO_QKINT"))


def _patch_act_tables():
    """Force every activation onto the one table set containing
    exp+ln+copy+identity, so the kernel performs a single ACT_TABLE_LOAD
    instead of thrashing between per-function sets (1.3us each)."""
    import functools
    from concourse import bacc as _bacc
    from concourse import hw_specs as _hw
    if getattr(_bacc, "_act_tables_patched", False):
        return
    orig = _hw.get_activation_tables

    @functools.cache
    def patched(arch):
        tabs = dict(orig(arch))
        full = "natural_log_exp_and_others"
        keep = tabs[full]
        strip = {f for f in keep}
        out = {}
        for name, funcs in tabs.items():
            out[name] = funcs if name == full else (funcs - strip)
        return out

    _bacc.get_activation_tables = patched
    _bacc._act_tables_patched = True


def _patch_ldw_opt():
    """Enable walrus's LDWEIGHTS optimization (dedupes/overlaps weight
    loads); concourse pins it off by default."""
    from concourse import bass_utils as _bu
    if getattr(_bu, "_ldw_opt_patched", False):
        return
    orig = _bu.run_command

    def patched(cmd, *a, **kw):
        cmd = ["--enable-ldw-opt=true" if c == "--enable-ldw-opt=false"
               else c for c in cmd]
        return orig(cmd, *a, **kw)

    _bu.run_command = patched
    _bu._ldw_opt_patched = True


def _build():
    _patch_act_tables()
    if os.environ.get("KB_LDWOPT"):
        _patch_ldw_opt()
    nc = bacc.Bacc("TRN2", target_bir_lowering=False, debug=False,
                   enable_asserts=False, num_devices=8)

    xT = nc.dram_tensor("xT", [DMODEL, NSEQ], F16, kind="ExternalInput").ap()
    maskT = nc.dram_tensor("maskT", [NSEQ, NSEQ], F16, kind="ExternalInput").ap()
    wq_d = nc.dram_tensor("wq", [DMODEL, HD], F16, kind="ExternalInput").ap()
    wk_d = nc.dram_tensor("wk", [DMODEL, HD], F16, kind="ExternalInput").ap()
    wv_d = nc.dram_tensor("wv", [DMODEL, HD], F16, kind="ExternalInput").ap()
    wo_d = nc.dram_tensor("wo", [P, NPAIR, DMODEL], F16, kind="ExternalInput").ap()
    bq_d = nc.dram_tensor("bq2", [P, NPAIR], F32, kind="ExternalInput").ap()
    bk_d = nc.dram_tensor("bk2", [P, NPAIR], F32, kind="ExternalInput").ap()
    bvr_d = nc.dram_tensor("bvr", [P, HD], F32, kind="ExternalInput").ap()
    ones_d = nc.dram_tensor("ones2", [P, 512], F16, kind="ExternalInput").ap()
    ones1_d = nc.dram_tensor("ones1", [P, 1], F16, kind="ExternalInput").ap()
    sel4_d = nc.dram_tensor("sel4", [97, P], F16, kind="ExternalInput").ap()
    y_d = nc.dram_tensor("y", [NSEQ, DMODEL], F32, kind="ExternalOutput").ap()

    xT_r = xT.rearrange("(dc p) n -> p dc n", p=P)        # [128, 8, 2048]
    maskT_r = maskT.rearrange("(kc p) q -> p kc q", p=P)  # [128, 16, 2048]

    with tile.TileContext(nc) as tc, ExitStack() as ctx:
        persist = ctx.enter_context(tc.tile_pool(name="persist", bufs=1))
        x = persist.tile([P, DC, NSEQ], F16)     # resident input (transposed)
        qT = persist.tile([P, NPAIR, NSEQ], F16)  # [hd%128, pair, seq]
        kT = persist.tile([P, NPAIR, NSEQ], F16)
        v = persist.tile([P, NKC, HD], F16)       # [seq%128, seq-chunk, hd]
        wq = persist.tile([P, DC, HD], F16)
        wk = persist.tile([P, DC, HD], F16)
        wv = persist.tile([P, DC, HD], F16)
        wo = persist.tile([P, NPAIR, DMODEL], F16)
        ones = persist.tile([P, 512], F16)
        ones1 = persist.tile([P, 1], F16)
        sel4 = persist.tile([97, P], F16)
        bqs = persist.tile([P, NPAIR], F32)
        bks = persist.tile([P, NPAIR], F32)
        bvr = persist.tile([P, HD], F32)

        nc.sync.dma_start(out=ones, in_=ones_d)
        nc.sync.dma_start(out=ones1, in_=ones1_d)
        nc.sync.dma_start(out=sel4, in_=sel4_d)
        nc.sync.dma_start(out=bqs, in_=bq_d)
        nc.sync.dma_start(out=bks, in_=bk_d)
        nc.sync.dma_start(out=bvr, in_=bvr_d)
        nc.sync.dma_start(out=wv, in_=wv_d.rearrange("(dc p) m -> p dc m", p=P))
        # stage x per seq-chunk so v-projection can start early
        for dc in range(DC):
            nc.sync.dma_start(out=x[:, dc, 0:512], in_=xT_r[:, dc, 0:512])
        nc.sync.dma_start(out=wq, in_=wq_d.rearrange("(dc p) m -> p dc m", p=P))
        nc.sync.dma_start(out=wk, in_=wk_d.rearrange("(dc p) m -> p dc m", p=P))
        for n in range(1, NQC):
            for dc in range(DC):
                nc.sync.dma_start(out=x[:, dc, n * 512:(n + 1) * 512],
                                  in_=xT_r[:, dc, n * 512:(n + 1) * 512])
        nc.sync.dma_start(out=wo, in_=wo_d)

        ssp = ctx.enter_context(tc.tile_pool(name="ssp", bufs=2, space="PSUM"))
        pcp = ctx.enter_context(tc.tile_pool(name="pcp", bufs=2, space="PSUM"))
        dnp = ctx.enter_context(tc.tile_pool(name="dnp", bufs=2, space="PSUM"))
        mkpool = ctx.enter_context(tc.tile_pool(name="mk", bufs=3))
        expool = ctx.enter_context(tc.tile_pool(name="ex", bufs=4))
        exmpool = ctx.enter_context(tc.tile_pool(name="exm", bufs=5))
        cxpool = ctx.enter_context(tc.tile_pool(name="cx", bufs=4))
        trpool = ctx.enter_context(tc.tile_pool(name="tr", bufs=2))
        ypool = ctx.enter_context(tc.tile_pool(name="yo", bufs=3))

        # ---- PE warmup (HAM) ----
        wt = ssp.tile([P, 2, 512], F32, tag="ss", name="warm")
        for i in range(16):
            nc.tensor.matmul(wt[:, i % 2, :], lhsT=ones[:, 0:P], rhs=ones,
                             start=(i < 2), stop=(i >= 14))

        # ---- v projection (all pairs at once, [seq, hd] layout) ----
        for n in range(NQC):
            for s2 in range(2):
                psv = ssp.tile([P, 2, 512], F32, tag="ss", name="psv")
                for j in range(2):
                    s = n * 4 + s2 * 2 + j
                    for dc in range(DC):
                        nc.tensor.matmul(
                            psv[:, j, :],
                            lhsT=x[:, dc, s * 128:(s + 1) * 128],
                            rhs=wv[:, dc, :],
                            start=(dc == 0), stop=(dc == DC - 1))
                for j in range(2):
                    s = n * 4 + s2 * 2 + j
                    nc.vector.tensor_tensor(v[:, s, :], psv[:, j, :], bvr,
                                            OP.add)

        # ---- q/k projection piece: one (q-or-k, seq-chunk) column ----
        def qk_piece(p, idx):
            w_sb, b_sb, dst = ((wq, bqs, qT), (wk, bks, kT))[idx // 4]
            n = idx % 4
            ps = ssp.tile([P, 2, 512], F32, tag="ss", name="qk")
            for dc in range(DC):
                nc.tensor.matmul(
                    ps[:, 0, :],
                    lhsT=w_sb[:, dc, p * 128:(p + 1) * 128],
                    rhs=x[:, dc, n * 512:(n + 1) * 512],
                    start=(dc == 0), stop=(dc == DC - 1))
            nc.scalar.activation(
                out=dst[:, p, n * 512:(n + 1) * 512], in_=ps[:, 0, :],
                func=AF.Identity, bias=b_sb[:, p:p + 1], scale=1.0)

        def qk_proj(p):
            for idx in range(8):
                qk_piece(p, idx)

        qk_proj(0)
        if _NO_QKINT:
            for p in range(1, NPAIR):
                qk_proj(p)

        # ---- output projection for one q-chunk quarter (qs) ----
        def outproj(qc, cps, qs):
            q0 = qc * 512
            py = ssp.tile([P, 2, 512], F32, tag="ss", name="py")
            for dm in range(2):
                for c in range(NPAIR):
                    nc.tensor.matmul(
                        py[:, dm, :],
                        lhsT=cps[c // 2][:, c % 2, qs * 128:(qs + 1) * 128],
                        rhs=wo[:, c, dm * 512:(dm + 1) * 512],
                        start=(c == 0), stop=(c == NPAIR - 1))
            ysb = ypool.tile([P, 2, 512], F32, tag="y")
            nc.scalar.activation(out=ysb, in_=py, func=AF.Copy,
                                 scale=1.0 / 16.0)
            nc.sync.dma_start(
                out=y_d[q0 + qs * 128:q0 + (qs + 1) * 128, :].rearrange(
                    "q (dm n) -> q dm n", n=512),
                in_=ysb)

        # ---- deferred normalize finish: dsum matmul + 1/d + ctx scale ----
        # (emitted inside the NEXT row's kc loop so the in-order PE queue
        # never blocks on the dns ACT copy at a pair boundary)
        def norm_finish(pend):
            dns_t, pc_t, cp2_t, slot, tag = pend
            dsum = dnp.tile([P, 512], F32, tag="dn", name=f"ds{tag}")
            nc.tensor.matmul(dsum, lhsT=sel4, rhs=dns_t,
                             start=True, stop=True)
            t2 = trpool.tile([P, 512], F32, tag="t2")
            r2 = trpool.tile([P, 512], F32, tag="r2")
            nc.scalar.activation(out=t2, in_=dsum, func=AF.Ln)
            nc.scalar.activation(out=r2, in_=t2, func=AF.Exp, scale=-1.0)
            nc.vector.tensor_tensor(cp2_t[:, slot, :], pc_t, r2, OP.mult)

        # ---- attention ----
        prev = None   # (qc, cps) awaiting deferred output projection
        pending = None  # normalize awaiting finish in the next row
        for qc in range(NQC):
            q0 = qc * 512
            mk_tiles = []
            for j in range(2):
                mk = mkpool.tile([P, 8, 512], F16, tag="mk")
                nc.sync.dma_start(
                    out=mk, in_=maskT_r[:, 8 * j:8 * j + 8, q0:q0 + 512])
                mk_tiles.append(mk)
            cps = []
            for p in range(NPAIR):
                dn = dnp.tile([P, 512], F32, tag="dn", name=f"dn{qc}_{p}")
                if qc == 0 and p < 2:
                    # first use of each dn ring slot: clear power-on garbage
                    # so ln() on rows 1..31 stays finite (1.0 -> ln=0)
                    nc.vector.memset(dn, 1.0)
                pc = pcp.tile([P, 512], F32, tag="pc", name=f"pc{qc}_{p}")
                exm = ex = None
                for kc in range(NKC):
                    ss = ssp.tile([P, 2, 512], F32, tag="ss", name="ss")
                    for hp in range(2):
                        nc.tensor.matmul(
                            ss[:, hp, :],
                            lhsT=kT[64 * hp:64 * hp + 64, p,
                                    kc * 128:(kc + 1) * 128],
                            rhs=qT[64 * hp:64 * hp + 64, p, q0:q0 + 512],
                            start=True, stop=True)
                    if kc % 2 == 0:
                        exm = exmpool.tile([P, 2, 2, 512], F16, tag="exm")
                    mk = mk_tiles[kc // 8]
                    mkb = mk[:, kc % 8, :].unsqueeze(1).broadcast_to(
                        (P, 2, 512))
                    nc.vector.tensor_tensor(exm[:, kc % 2, :, :], ss, mkb,
                                            OP.mult)
                    if kc % 2 == 1:
                        ex = expool.tile([P, 2, 2, 512], F16, tag="ex")
                        nc.scalar.activation(out=ex, in_=exm, func=AF.Exp,
                                             scale=0.125)
                        for j in range(2):
                            kcj = kc - 1 + j
                            for hp in range(2):
                                nc.tensor.matmul(
                                    pc[64 * hp:64 * hp + 64, :],
                                    lhsT=v[:, kcj,
                                           p * 128 + 64 * hp:
                                           p * 128 + 64 * hp + 64],
                                    rhs=ex[:, j, hp, :],
                                    start=(kcj == 0), stop=(kcj == NKC - 1),
                                    tile_position=(0, 64 * hp))
                        if not _NO_DENOM:
                            # 4-slot partial denominators: one col-packed
                            # pass per 2 kc (slots 0/32/64/96)
                            for j in range(2):
                                for hp in range(2):
                                    s = 32 * (2 * j + hp)
                                    nc.tensor.matmul(
                                        dn[s:s + 1, :],
                                        lhsT=ones1[:, 0:1],
                                        rhs=ex[:, j, hp, :],
                                        start=(kc == 1),
                                        stop=(kc == NKC - 1),
                                        tile_position=(0, s))
                    # finish the previous pair's normalize early this row
                    if kc == 1 and pending is not None:
                        norm_finish(pending)
                        pending = None
                    # interleave next pair's q/k projection into qc0 rows
                    if (qc == 0 and p < 3 and not _NO_QKINT
                            and kc % 2 == 1):
                        qk_piece(p + 1, (kc - 1) // 2)
                    # interleave previous q-chunk's output projection into
                    # this q-chunk's first row (keeps DVE fed at boundaries)
                    if p == 0 and prev is not None and kc % 4 == 2:
                        outproj(prev[0], prev[1], kc // 4)
                        if kc == 14:
                            prev = None
                # normalize this pair (two pairs share one cx tile)
                if p % 2 == 0:
                    cp2 = cxpool.tile([P, 2, 512], F16, tag="cx")
                    cps.append(cp2)
                if _NO_DENOM or _NO_NORM:
                    nc.vector.tensor_copy(out=cp2[:, p % 2, :], in_=pc)
                else:
                    # prep now (frees the dn bank); defer the dsum matmul +
                    # 1/d + ctx scale into the next row's kc loop
                    dns = trpool.tile([97, 512], F16, tag="dns")
                    nc.scalar.activation(out=dns, in_=dn[0:97, :],
                                         func=AF.Copy, scale=1.0 / 16.0)
                    pending = (dns, pc, cp2, p % 2, f"{qc}_{p}")
            prev = (qc, cps)
        if pending is not None:
            norm_finish(pending)
            pending = None
        # final q chunk's output projection (nothing left to overlap)
        for qs in range(4):
            outproj(prev[0], prev[1], qs)
    nc.compile()
    return nc


def _get_nc():
    if "nc" not in _CACHE:
        _CACHE["nc"] = _build()
    return _CACHE["nc"]


def kernel(input, mask, Wq, bq, Wk, bk, Wv, bv, Wo, bo):
    x = np.asarray(input, dtype=np.float32)
    m = np.asarray(mask, dtype=np.float32)
    Wq = np.asarray(Wq, dtype=np.float32)
    Wk = np.asarray(Wk, dtype=np.float32)
    Wv = np.asarray(Wv, dtype=np.float32)
    Wo = np.asarray(Wo, dtype=np.float32)
    bq = np.asarray(bq, dtype=np.float32)
    bk = np.asarray(bk, dtype=np.float32)
    bv = np.asarray(bv, dtype=np.float32)
    bo = np.asarray(bo, dtype=np.float32)
    B = x.shape[0]
    assert x.shape == (B, NSEQ, DMODEL) and B == 4

    sel4 = np.zeros((97, P), np.float16)
    sel4[0, 0:64] = 1.0
    sel4[64, 0:64] = 1.0
    sel4[32, 64:128] = 1.0
    sel4[96, 64:128] = 1.0
    f8 = mybir.dt.np(F8)

    nc = _get_nc()
    in_maps = []
    for b in range(B):
        xT = np.ascontiguousarray(x[b].T)
        mT = np.ascontiguousarray(m[b].T)
        for hg in range(2):
            sl = slice(hg * HD, (hg + 1) * HD)
            in_maps.append({
                "xT": xT.astype(np.float16),
                "maskT": mT.astype(np.float16),
                "wq": np.ascontiguousarray(Wq[:, sl]).astype(np.float16),
                "wk": np.ascontiguousarray(Wk[:, sl]).astype(np.float16),
                "wv": np.ascontiguousarray(Wv[:, sl]).astype(np.float16),
                "wo": np.ascontiguousarray(
                    Wo[sl].reshape(NPAIR, P, DMODEL).transpose(1, 0, 2)
                ).astype(np.float16),
                "bq2": np.ascontiguousarray(bq[sl].reshape(NPAIR, P).T),
                "bk2": np.ascontiguousarray(bk[sl].reshape(NPAIR, P).T),
                "bvr": np.ascontiguousarray(
                    np.broadcast_to(bv[sl], (P, HD))),
                "ones2": np.ones((P, 512), dtype=np.float16),
                "ones1": np.ones((P, 1), dtype=np.float16),
                "sel4": sel4,
            })

    res = bass_utils.run_bass_kernel_spmd(nc, in_maps, core_ids=list(range(8)))
    global LAST_RESULTS
    LAST_RESULTS = res

    out = np.empty((B, NSEQ, DMODEL), dtype=np.float32)
    for b in range(B):
        out[b] = res.results[2 * b]["y"] + res.results[2 * b + 1]["y"] + bo
    return out
